# revision 10
# baseline (speedup 1.0000x reference)
"""MatchLSTM attention kernel for 8 Trainium2 NeuronCores.

Reference computation (B=64, T=2048, D=512):
    G   = tanh(input_p@Wp.T + bp + input_q@Wq.T + bq + h_tm1@Wr.T + br)
    a   = softmax(G@w + match_b)            over T
    z   = sum_t a[:,t] * input_q[:,:,t]
    out = concat([input_p, z], -1)

Sharding: data-parallel over batch, 8 batches per core, weights replicated.

Per-core pipeline (ACT-bound: tanh of G is the irreducible cost):
  - c[b,o] = input_p@Wp.T + h@Wr.T + (bp+bq+br) computed on HOST in fp32,
    uploaded as the per-partition tanh bias.  match_b dropped (softmax
    shift-invariant).
  - EVERY PE matmul uses fp8e4m3 DoubleRow (contraction 256 per call,
    0.5 cycles/out-col).  Mixing DoubleRow with normal-mode matmuls was
    observed to corrupt PE results non-deterministically on hw, so the
    kernel keeps the PE in one mode throughout:
      * G^T[o,t] tiles [128,1024]: stationary Wq^T chunk, moving X^T.
      * scores: tanh-pair tile [128o,2,128t] STATIONARY, w pair-column
        moving -> sT column [128t,1] directly transposed; a second pass
        with the fp8 residual of w cancels w's quantization error.
      * z: xnat chunk-pair [128t,2,128q] STATIONARY, esc column pair
        moving -> z[128q,1].
  - tanh fused with bias on ScalarE -> fp8 SBUF, oc-pairs sharing one
    [128,2,1024] tile; exp once per batch ([128,16]) -> fp8 esc with
    per-partition sumexp via accum_out.
  - Raw z and pesum are DMA'd out; the host performs the 1/sumexp scale
    (bit-equivalent fp32 divide, removes all non-DoubleRow PE work).
  - Score matmuls are emitted with a one-tile lag and batch tails with a
    two-tile lag so the in-order PE queue never blocks the ACT engine.
"""

import sys

if "/opt/trn_rl_repo" not in sys.path:
    sys.path.insert(0, "/opt/trn_rl_repo")

import numpy as np
import ml_dtypes

N_CORES = 8
B, T, D = 64, 2048, 512
PB = B // N_CORES          # batches per core
NJ = T // 128              # 16 token chunks of 128 (esc/xnat granularity)

BF16 = ml_dtypes.bfloat16
FP8 = ml_dtypes.float8_e4m3

_CACHE: dict = {}


def _build_program():
    import concourse.bacc as bacc
    import concourse.tile as tile
    import concourse.mybir as mybir
    from concourse.bass import MemorySpace

    dt = mybir.dt
    F32 = dt.float32
    F8 = dt.float8e4
    AF = mybir.ActivationFunctionType
    DR = mybir.MatmulPerfMode.DoubleRow

    nc = bacc.Bacc(
        "TRN2", target_bir_lowering=False, debug=False, num_devices=N_CORES
    )

    # dram inputs (host-prepared layouts, all DMAs are contiguous copies)
    xqT_d = nc.dram_tensor("xqT", [PB, 128, 2, 2, T], F8, kind="ExternalInput")
    xnat_d = nc.dram_tensor("xnat", [PB, 128, NJ, 512], F8, kind="ExternalInput")
    wq_d = nc.dram_tensor("wqt", [128, 2, 2, D], F8, kind="ExternalInput")
    ct_d = nc.dram_tensor("ct", [128, 4, PB], F32, kind="ExternalInput")
    # w split into fp8 main + fp8 residual, laid out as [p, ocpair, u, 16]
    # (padded so the DR pair-dim stride is 16 elements, an ISA requirement)
    wcol_d = nc.dram_tensor("wcol", [128, 2, 2, 32], F8, kind="ExternalInput")
    z_d = nc.dram_tensor("z", [PB, 128, 4], F32, kind="ExternalOutput")
    p_d = nc.dram_tensor("pe", [PB, 128, 1], F32, kind="ExternalOutput")

    with tile.TileContext(nc) as tc:
        with (
            tc.tile_pool(name="consts", bufs=1) as consts,
            tc.tile_pool(name="xT_p", bufs=3) as xT_pool,
            tc.tile_pool(name="xnat_p", bufs=3) as xnat_pool,
            tc.tile_pool(name="th_p", bufs=3) as th_pool,
            tc.tile_pool(name="esc_p", bufs=2) as esc_pool,
            tc.tile_pool(name="small_p", bufs=2) as small_pool,
            tc.tile_pool(name="pG", bufs=2, space=MemorySpace.PSUM) as pG,
            tc.tile_pool(name="pST", bufs=1, space=MemorySpace.PSUM) as pST,
            tc.tile_pool(name="pZ", bufs=1, space=MemorySpace.PSUM) as pZ,
        ):
            # ---- constants (DMA order = criticality order) -----------------
            wq_s = consts.tile([128, 2, 2, D], F8, tag="wq", name="wq_s")
            nc.sync.dma_start(out=wq_s, in_=wq_d[:, :, :, :])
            cT_s = consts.tile([128, 4, PB], F32, tag="cT", name="cT_s")
            nc.sync.dma_start(out=cT_s, in_=ct_d[:, :, :])
            wcol_s = consts.tile([128, 2, 2, 32], F8, tag="wcol", name="wcol_s")
            nc.sync.dma_start(out=wcol_s, in_=wcol_d[:, :, :, :])

            # per-batch state captured across the lagged emission stream
            st: dict = {}

            def batch_start(b):
                xT = xT_pool.tile([128, 2, 2, T], F8, tag="xT", name="xT")
                # split the transfer so the first token-half lands sooner
                for h in range(2):
                    nc.sync.dma_start(
                        out=xT[:, :, :, h * 1024 : (h + 1) * 1024],
                        in_=xqT_d[b, :, :, :, h * 1024 : (h + 1) * 1024],
                    )
                xnat = xnat_pool.tile([128, NJ, 512], F8, tag="xnat", name="xnat")
                nc.sync.dma_start(out=xnat, in_=xnat_d[b])
                esc = esc_pool.tile([128, NJ // 2, 2, 16], F8, tag="esc", name="esc")
                nc.vector.memset(esc, 0.0)  # pad cols must be 0 for the z matmul
                s_sb = small_pool.tile([128, NJ // 2, 2], F32, tag="ssb", name="s_sb")
                pesum = small_pool.tile([128, 1], F32, tag="pesum", name="pesum")
                st[b] = dict(xT=xT, xnat=xnat, s_sb=s_sb, esc=esc, pesum=pesum)

            def emit_g(b, h, oc):
                xT = st[b]["xT"]
                g_ps = pG.tile([128, 1024], F32, tag="g", name="g_ps")
                for g2 in range(2):
                    for i in range(2):
                        t0 = h * 1024 + i * 512
                        nc.tensor.matmul(
                            g_ps[:, i * 512 : (i + 1) * 512],
                            wq_s[:, g2, :, oc * 128 : (oc + 1) * 128],
                            xT[:, g2, :, t0 : t0 + 512],
                            start=(g2 == 0),
                            stop=(g2 == 1),
                            perf_mode=DR,
                        )
                return g_ps

            def emit_tanh(b, h, oc, g_ps):
                # oc-pairs share one [128, 2, 1024] fp8 tile (DR stationary)
                if oc % 2 == 0:
                    st[b]["th2"] = th_pool.tile(
                        [128, 2, 1024], F8, tag="th", name="th2"
                    )
                th2 = st[b]["th2"]
                nc.scalar.activation(
                    out=th2[:, oc % 2, :],
                    in_=g_ps,
                    func=AF.Tanh,
                    bias=cT_s[:, oc, b : b + 1],
                    scale=1.0,
                )
                return th2

            def emit_scores(b, h, oc, th2):
                # called after the odd-oc tanh of pair ocp = oc // 2.
                # PSUM session rule: a start=True resets the bank's
                # written-bitmap, so a group's accumulating writes must all
                # happen before any other start targets the bank.  Each
                # (jj, ocp) group is a back-to-back start/stop pair into its
                # own region; the two ocp halves are summed on DVE.
                ocp = oc // 2
                if ocp == 0 and "sT" not in st[b]:
                    st[b]["sT"] = pST.tile([128, 8, 2, 16], F32, tag="st", name="sT_ps")
                sT_ps = st[b]["sT"]
                for jj in range(8):
                    for r in range(2):  # w main + residual
                        nc.tensor.matmul(
                            sT_ps[:, jj, ocp, :],
                            th2[:, :, jj * 128 : (jj + 1) * 128],
                            wcol_s[:, ocp, :, 16 * r : 16 * r + 16],
                            start=(r == 0),
                            stop=(r == 1),
                            perf_mode=DR,
                        )
                if ocp == 1:
                    # s = ocp0 partial + ocp1 partial (DVE reads at most one
                    # PSUM operand, so stage ocp0 through SBUF); evacuating
                    # frees the bank for the other half
                    s_sb = st[b]["s_sb"]
                    j0 = h * 8
                    s_tmp = small_pool.tile([128, 8], F32, tag="stmp", name="s_tmp")
                    nc.vector.tensor_copy(out=s_tmp, in_=sT_ps[:, :, 0, 0])
                    nc.vector.tensor_add(
                        s_sb.rearrange("p m u -> p (m u)")[:, j0 : j0 + 8],
                        s_tmp,
                        sT_ps[:, :, 1, 0],
                    )
                    st[b].pop("sT")

            def batch_tail(b):
                s = st.pop(b)
                esc, pesum, xnat = s["esc"], s["pesum"], s["xnat"]
                # exp of all 16 score columns; per-partition sumexp for free
                nc.scalar.activation(
                    out=esc[:, :, :, 0],
                    in_=s["s_sb"],
                    func=AF.Exp,
                    bias=0.0,
                    scale=1.0,
                    accum_out=pesum,
                )
                # z[q] = sum_t esc_t * X[t, q]  (xnat chunk-pair stationary)
                z_ps = pZ.tile([128, 4, 128], F32, tag="z", name="z_ps")
                for qc in range(4):
                    for m in range(NJ // 2):
                        nc.tensor.matmul(
                            z_ps[:, qc, 0:16],
                            xnat[:, 2 * m : 2 * m + 2, qc * 128 : (qc + 1) * 128],
                            esc[:, m, :, :],
                            start=(m == 0),
                            stop=(m == NJ // 2 - 1),
                            perf_mode=DR,
                        )
                zr = small_pool.tile([128, 4], F32, tag="zr", name="zr")
                nc.vector.tensor_copy(out=zr, in_=z_ps[:, :, 0])

                nc.sync.dma_start(out=z_d[b], in_=zr)
                nc.sync.dma_start(out=p_d[b], in_=pesum)

            # ---- lagged emission stream -----------------------------------
            tiles = [(b, h, oc) for b in range(PB) for h in range(2) for oc in range(4)]
            NTILES = len(tiles)
            pending: dict = {}
            for idx in range(NTILES + 2):
                if idx < NTILES:
                    b, h, oc = tiles[idx]
                    if h == 0 and oc == 0:
                        batch_start(b)
                    g_ps = emit_g(b, h, oc)
                # batch tail with two-tile lag (keeps ACT fed at boundaries)
                if idx >= 2:
                    pb_, ph_, poc_ = tiles[idx - 2]
                    if ph_ == 1 and poc_ == 3:
                        batch_tail(pb_)
                # scores with one-tile lag, after each odd-oc tanh
                if 1 <= idx <= NTILES:
                    pb_, ph_, poc_ = tiles[idx - 1]
                    if poc_ % 2 == 1:
                        emit_scores(pb_, ph_, poc_, pending.pop(idx - 1))
                if idx < NTILES:
                    pending[idx] = emit_tanh(b, h, oc, g_ps)

    nc.compile()
    return nc


def _get_program():
    if "nc" not in _CACHE:
        _CACHE["nc"] = _build_program()
    return _CACHE["nc"]


def kernel(**inputs) -> np.ndarray:
    from concourse import bass_utils

    inp = {k: np.asarray(v) for k, v in inputs.items()}
    input_p = inp["input_p"].astype(np.float32)
    input_q = inp["input_q"].astype(np.float32)
    h_tm1 = inp["h_tm1"].astype(np.float32)
    Wp, Wq, Wr = inp["Wp"], inp["Wq"], inp["Wr"]
    bp, bq, br = inp["bp"], inp["bq"], inp["br"]
    w = np.asarray(inp["w"], dtype=np.float32)
    # match_b is a constant shift of the pre-softmax scores: softmax-invariant.

    # shared (weight) tensors
    wqt = np.ascontiguousarray(
        Wq.T.reshape(2, 2, 128, D).transpose(2, 0, 1, 3)
    ).astype(FP8)
    # w as fp8 main + fp8 residual (second DR pass cancels quantization)
    w8 = w.astype(FP8)
    wres = (w - w8.astype(np.float32)).astype(FP8)
    wcol = np.zeros((128, 2, 2, 32), dtype=FP8)
    wcol[:, :, :, 0] = w8.reshape(2, 2, 128).transpose(2, 0, 1)
    wcol[:, :, :, 16] = wres.reshape(2, 2, 128).transpose(2, 0, 1)
    # c[b,o] = input_p@Wp.T + h@Wr.T + (bp+bq+br), fp32 on host
    c = (
        input_p @ Wp.T.astype(np.float32)
        + h_tm1 @ Wr.T.astype(np.float32)
        + (bp + bq + br).astype(np.float32)
    )

    nc = _get_program()

    in_maps = []
    for cix in range(N_CORES):
        s = slice(cix * PB, (cix + 1) * PB)
        xq = input_q[s]  # (PB, T, D)
        xqT = np.ascontiguousarray(
            xq.transpose(0, 2, 1).reshape(PB, 2, 2, 128, T).transpose(0, 3, 1, 2, 4)
        ).astype(FP8)
        xnat = np.ascontiguousarray(
            xq.reshape(PB, NJ, 128, D).transpose(0, 2, 1, 3)
        ).astype(FP8)
        ct = np.ascontiguousarray(
            c[s].T.reshape(4, 128, PB).transpose(1, 0, 2)
        ).astype(np.float32)
        in_maps.append(
            {"xqT": xqT, "xnat": xnat, "wqt": wqt, "ct": ct, "wcol": wcol}
        )

    res = bass_utils.run_bass_kernel_spmd(
        nc, in_maps, core_ids=list(range(N_CORES))
    )
    zs = []
    for cix in range(N_CORES):
        zraw = np.asarray(res.results[cix]["z"], dtype=np.float32)   # [PB,128,4]
        pes = np.asarray(res.results[cix]["pe"], dtype=np.float32)   # [PB,128,1]
        S = pes[:, :, 0].sum(axis=1)                                  # [PB]
        zs.append(
            (zraw.transpose(0, 2, 1).reshape(PB, D) / S[:, None]).astype(np.float32)
        )
    z = np.concatenate(zs, axis=0)
    return np.concatenate([input_p, z], axis=1)
